# revision 1
# baseline (speedup 1.0000x reference)
"""Two-layer GAT (PyG GATConv semantics) on 8 Trainium2 NeuronCores.

Strategy (edge-parallel, per the sharding hint):
  - self-loops added, edges sorted by dst; dst space split into 8
    contiguous, edge-balanced ranges (one per core), with the rank-3/4
    boundary pinned at NHALF so layer-1 and layer-2 gather-table halves
    share one edge grouping.
  - per core, dst segments are packed into uniform "groups" of 1024 edge
    slots (tiles 0-3: src < NHALF, tiles 4-7: src >= NHALF, <=128
    distinct dst nodes). Pad slots point at row 0 ("parked") and carry
    all-zero indicator columns, so they contribute nothing.
  - node phase (replicated on every core): h1|a_src|a_dst for all nodes
    via one matmul per 128-node tile against W1_ext = [W1 | W1@Asrc |
    W1@Adst]; rows written to bf16 gather tables (two halves, int16
    indexable) and a_src/a_dst scattered into a core-local table.
  - edge phase: dma_gather rows by src, gather a_dst by dst (local
    table), p = exp(leaky(as+ad)) in-place, msg *= p, segmented softmax
    numerator+denominator via one-hot-indicator matmuls accumulated in
    PSUM, normalize (+relu), h2 = out1 @ W2_ext via PE transpose,
    scatter [h2|as2|ad2] rows into the core's node-space shard.
  - one AllGather of the (padded, equal-size) shards between layers;
    layer 2 repeats the edge phase on the gathered table and scatters
    final rows into the output shard. Host concatenates shards.
"""
import math
import numpy as np

P = 128

F32 = None              # filled on first bass import
BF16 = None
I32 = None
I16 = None


# --------------------------------------------------------------------------
# configuration
# --------------------------------------------------------------------------
class Cfg:
    def __init__(self, N, IN, HID, H1, OUT, ncores=8, G=16):
        self.N, self.IN, self.HID, self.H1, self.OUT = N, IN, HID, H1, OUT
        self.ncores = ncores
        self.HC1 = H1 * HID                  # 256
        nt = (N + P - 1) // P
        nt += nt % 2
        self.NODET = nt                      # node tiles (even)
        self.NPAD = nt * P
        self.NHALF = self.NPAD // 2          # table-half row count
        self.ROW1 = 3 * P                    # 384: [h(256)|as(4)|ad(4)|pad]
        assert self.HC1 + 2 * H1 <= self.ROW1
        self.ROW2 = P                        # 128: [h2(64)|as2(1)|ad2(1)|pad]
        assert OUT + 2 <= self.ROW2
        self.ADROW = P                       # adt_local row width (bf16)
        self.G = G                           # tiles per group
        self.GE = G * P
        self.HLOW = self.GE // 2


FULL = Cfg(N=50000, IN=128, HID=64, H1=4, OUT=64)


# --------------------------------------------------------------------------
# host-side edge preprocessing
# --------------------------------------------------------------------------
def wrap16(a):
    """flat idx array [n] -> dma_gather layout [128, n//16] (int16)."""
    n = a.size
    assert n % 16 == 0
    return np.tile(np.ascontiguousarray(a.reshape(n // 16, 16).T), (8, 1))


def prep_edges(edge_index, cfg):
    N, NC, NHALF = cfg.N, cfg.ncores, cfg.NHALF
    G, GE, HLOW = cfg.G, cfg.GE, cfg.HLOW
    src = np.concatenate([np.asarray(edge_index[0], np.int64),
                          np.arange(N, dtype=np.int64)])
    dst = np.concatenate([np.asarray(edge_index[1], np.int64),
                          np.arange(N, dtype=np.int64)])
    order = np.argsort(dst, kind="stable")
    src, dst = src[order], dst[order]
    Etot = src.size

    counts = np.bincount(dst, minlength=N)
    cum = np.cumsum(counts)
    seg_start = np.concatenate([[0], cum]).astype(np.int64)

    # dst ranges: bounds[NC//2] pinned at NHALF; edge-balanced within halves
    half_edges = int(cum[min(NHALF, N) - 1])
    bounds = [0]
    for c in range(1, NC // 2):
        bounds.append(int(np.searchsorted(cum, half_edges * c / (NC // 2))))
    bounds.append(min(NHALF, N))
    rest = Etot - half_edges
    for c in range(1, NC // 2):
        bounds.append(int(np.searchsorted(cum, half_edges + rest * c / (NC // 2))))
    bounds.append(N)
    node_ranges = [(bounds[i], bounds[i + 1]) for i in range(NC)]
    maxn = max(b - a for a, b in node_ranges)
    SHARDR = maxn + 1                       # + trash row
    assert (NC // 2) * SHARDR <= 32767, SHARDR
    rank_of = np.zeros(N, dtype=np.int64)
    b_arr = np.asarray(bounds)
    for c, (a, b) in enumerate(node_ranges):
        rank_of[a:b] = c

    # pack segments into groups per core
    core_groups = []
    for c, (n0, n1) in enumerate(node_ranges):
        groups = []          # list of (dlist, lo_edges, hi_edges, lo_per_d, hi_per_d)
        dlist, lo, hi = [], [], []
        for d in range(n0, n1):
            s, e = int(seg_start[d]), int(seg_start[d] + counts[d])
            es = src[s:e]
            elo = es[es < NHALF]
            ehi = es[es >= NHALF]
            if (len(lo) + elo.size > HLOW or len(hi) + ehi.size > HLOW
                    or len(dlist) >= P):
                groups.append((dlist, lo, hi))
                dlist, lo, hi = [], [], []
            dlist.append(d)
            lo.extend([(int(x), len(dlist) - 1) for x in elo])
            hi.extend([(int(x), len(dlist) - 1) for x in ehi])
        if dlist:
            groups.append((dlist, lo, hi))
        core_groups.append(groups)

    NG = max(len(g) for g in core_groups)

    per_core = []
    for c, (n0, n1) in enumerate(node_ranges):
        groups = core_groups[c]
        idx1lo = np.zeros((NG, HLOW), np.int64)
        idx1hi = np.zeros((NG, HLOW), np.int64)
        idx2lo = np.zeros((NG, HLOW), np.int64)
        idx2hi = np.zeros((NG, HLOW), np.int64)
        idxad = np.zeros((NG, GE), np.int64)
        ind = np.zeros((NG, GE, P), np.float32)
        scat = np.full((NG, P), maxn, np.int64)
        for g, (dlist, lo, hi) in enumerate(groups):
            scat[g, :len(dlist)] = np.asarray(dlist, np.int64) - n0
            for j, (s, li) in enumerate(lo):
                idx1lo[g, j] = s
                idx2lo[g, j] = rank_of[s] * SHARDR + (s - b_arr[rank_of[s]])
                idxad[g, j] = dlist[li] - n0
                ind[g, j, li] = 1.0
            for j, (s, li) in enumerate(hi):
                idx1hi[g, j] = s - NHALF
                idx2hi[g, j] = (rank_of[s] - NC // 2) * SHARDR + (s - b_arr[rank_of[s]])
                idxad[g, HLOW + j] = dlist[li] - n0
                ind[g, HLOW + j, li] = 1.0

        def w512(a):   # [NG, 512] -> [128, NG*32] int16
            return np.concatenate([wrap16(a[g]) for g in range(NG)], axis=1).astype(np.int16)

        def w1024(a):  # [NG, 1024] -> [128, NG*64] int16
            return np.concatenate([wrap16(a[g]) for g in range(NG)], axis=1).astype(np.int16)

        import ml_dtypes
        per_core.append({
            "idx1lo": w512(idx1lo), "idx1hi": w512(idx1hi),
            "idx2lo": w512(idx2lo), "idx2hi": w512(idx2hi),
            "idxad": w1024(idxad),
            # ind: [NG, GE, P] -> [128(edge p), NG*G*128 (g, t, m)]
            "ind": np.ascontiguousarray(
                ind.reshape(NG, G, P, P).transpose(2, 0, 1, 3)
                   .reshape(P, NG * G * P)).astype(ml_dtypes.bfloat16),
            "scat": np.ascontiguousarray(scat.T).astype(np.int32),  # [P, NG]
            "n0": n0, "n1": n1,
        })

    # adt scatter indices: node tile nt, partition p -> local row or trash
    nodes = np.arange(cfg.NPAD, dtype=np.int64)
    adt_sidx = []
    for c, (n0, n1) in enumerate(node_ranges):
        s = np.where((nodes >= n0) & (nodes < n1), nodes - n0, maxn)
        adt_sidx.append(np.ascontiguousarray(
            s.reshape(cfg.NODET, P).T).astype(np.int32))   # [P, NODET]

    return {
        "node_ranges": node_ranges, "maxn": maxn, "SHARDR": SHARDR,
        "NG": NG, "per_core": per_core, "adt_sidx": adt_sidx,
    }


def make_weights(inputs, cfg):
    H1, HID, HC1, OUT = cfg.H1, cfg.HID, cfg.HC1, cfg.OUT
    W1 = np.asarray(inputs["W1"], np.float32)
    W2 = np.asarray(inputs["W2"], np.float32)
    a_s1 = np.asarray(inputs["att_src1"], np.float32)
    a_d1 = np.asarray(inputs["att_dst1"], np.float32)
    a_s2 = np.asarray(inputs["att_src2"], np.float32)
    a_d2 = np.asarray(inputs["att_dst2"], np.float32)
    A_src = np.zeros((HC1, H1), np.float32)
    A_dst = np.zeros((HC1, H1), np.float32)
    for h in range(H1):
        A_src[h * HID:(h + 1) * HID, h] = a_s1[h]
        A_dst[h * HID:(h + 1) * HID, h] = a_d1[h]
    W1_ext = np.concatenate([W1, W1 @ A_src, W1 @ A_dst], axis=1)   # [IN, HC1+2H]
    W2_ext = np.concatenate([W2, W2 @ a_s2[0][:, None],
                             W2 @ a_d2[0][:, None]], axis=1)        # [HC1, OUT+2]
    return W1_ext.astype(np.float32), W2_ext.astype(np.float32)


# --------------------------------------------------------------------------
# bass kernel builder
# --------------------------------------------------------------------------
def build_kernel(cfg, NG, SHARDR, phases=("node", "l1", "ag", "l2"), repeat=1, l1parts=7,
                 nq=4, scratch=65536, single_packet=False):
    import concourse.bass as bass
    import concourse.bacc as bacc
    import concourse.mybir as mybir
    from concourse.tile import TileContext
    from concourse.masks import make_identity
    from concourse import library_config

    F32, BF, I32, I16 = (mybir.dt.float32, mybir.dt.bfloat16,
                         mybir.dt.int32, mybir.dt.int16)
    NC = cfg.ncores
    G, GE, HLOW = cfg.G, cfg.GE, cfg.HLOW
    IW, AW = HLOW // 16, GE // 16
    HC1, H1, OUT = cfg.HC1, cfg.H1, cfg.OUT
    EXTC = HC1 + 2 * H1                  # node-phase matmul output cols (264)
    NODET, NHALF, ROW1, ROW2 = cfg.NODET, cfg.NHALF, cfg.ROW1, cfg.ROW2
    HT = NODET // 2                      # node tiles per half

    nc = bacc.Bacc(num_swdge_queues=nq, dynamic_dma_scratch_size=scratch)

    xT_in = nc.declare_dram_parameter("xT", [P, cfg.NPAD], F32, isOutput=False)
    w1e_in = nc.declare_dram_parameter("w1e", [P, EXTC], F32, isOutput=False)
    w2e_in = nc.declare_dram_parameter("w2e", [2, P, OUT + 2], BF, isOutput=False)
    i1lo_in = nc.declare_dram_parameter("idx1lo", [P, NG * IW], I16, isOutput=False)
    i1hi_in = nc.declare_dram_parameter("idx1hi", [P, NG * IW], I16, isOutput=False)
    i2lo_in = nc.declare_dram_parameter("idx2lo", [P, NG * IW], I16, isOutput=False)
    i2hi_in = nc.declare_dram_parameter("idx2hi", [P, NG * IW], I16, isOutput=False)
    iad_in = nc.declare_dram_parameter("idxad", [P, NG * AW], I16, isOutput=False)
    ind_in = nc.declare_dram_parameter("ind", [P, NG * GE], BF, isOutput=False)
    scat_in = nc.declare_dram_parameter("scat", [P, NG], I32, isOutput=False)
    asx_in = nc.declare_dram_parameter("adt_sidx", [P, NODET], I32, isOutput=False)
    out_sh = nc.declare_dram_parameter("out_shard", [SHARDR, OUT], F32, isOutput=True)

    t1a = nc.dram_tensor("t1a", [NHALF, ROW1], BF)
    t1b = nc.dram_tensor("t1b", [NHALF, ROW1], BF)
    adt = nc.dram_tensor("adt_local", [SHARDR, cfg.ADROW], BF)
    t2s = nc.dram_tensor("t2_shard", [SHARDR, ROW2], BF)
    t2f = nc.dram_tensor("t2_full", [NC * SHARDR, ROW2], BF, addr_space="Shared")

    with TileContext(nc) as tc:
        with tc.tile_pool(name="const", bufs=1) as cpool:
            nc.gpsimd.load_library(library_config.mlp)
            ident = cpool.tile([P, P], BF)
            make_identity(nc, ident[:])
            w1e = cpool.tile([P, EXTC], F32)
            nc.sync.dma_start(out=w1e[:], in_=w1e_in[:])
            w2e = [cpool.tile([P, OUT + 2], BF, name=f"w2e{k}") for k in range(2)]
            nc.sync.dma_start(out=w2e[0][:], in_=w2e_in[0])
            nc.sync.dma_start(out=w2e[1][:], in_=w2e_in[1])
            i1lo = cpool.tile([P, NG * IW], I16)
            i1hi = cpool.tile([P, NG * IW], I16)
            i2lo = cpool.tile([P, NG * IW], I16)
            i2hi = cpool.tile([P, NG * IW], I16)
            iad = cpool.tile([P, NG * AW], I16)
            scat = cpool.tile([P, NG], I32)
            asx = cpool.tile([P, NODET], I32)
            for t, src_t in ((i1lo, i1lo_in), (i1hi, i1hi_in), (i2lo, i2lo_in),
                             (i2hi, i2hi_in), (iad, iad_in), (scat, scat_in),
                             (asx, asx_in)):
                nc.sync.dma_start(out=t[:], in_=src_t[:])

            # ---------------- node phase (replicated) ----------------
            def phase_node():
                with tc.tile_pool(name="xph", bufs=3) as xpool, \
                     tc.tile_pool(name="hps", bufs=4, space="PSUM") as hpp, \
                     tc.tile_pool(name="rows", bufs=3) as rpool:
                    for nt in range(NODET):
                        xt = xpool.tile([P, P], F32)
                        nc.sync.dma_start(out=xt[:], in_=xT_in[:, nt * P:(nt + 1) * P])
                        hp = hpp.tile([P, EXTC], F32, space="PSUM")
                        nc.tensor.matmul(out=hp[:], lhsT=xt[:], rhs=w1e[:],
                                         start=True, stop=True)
                        row = rpool.tile([P, EXTC], BF)
                        nc.scalar.activation(out=row[:], in_=hp[:],
                                             func=mybir.ActivationFunctionType.Copy)
                        tdst = t1a if nt < HT else t1b
                        r0 = (nt % HT) * P
                        nc.sync.dma_start(out=tdst[r0:r0 + P, 0:EXTC], in_=row[:])
                        nc.gpsimd.indirect_dma_start(
                            out=adt[:, :],
                            out_offset=bass.IndirectOffsetOnAxis(
                                ap=asx[:, nt:nt + 1], axis=0),
                            in_=row[:, HC1:HC1 + 2 * H1], in_offset=None)

            # ---------------- layer-1 edge phase ----------------
            def phase_l1():
                with tc.tile_pool(name="gt", bufs=3) as gtp, \
                     tc.tile_pool(name="adg", bufs=3) as adp, \
                     tc.tile_pool(name="indp", bufs=3) as indp, \
                     tc.tile_pool(name="ps1", bufs=2, space="PSUM") as psp, \
                     tc.tile_pool(name="tp", bufs=2, space="PSUM") as tpp, \
                     tc.tile_pool(name="h2p", bufs=2, space="PSUM") as h2pp, \
                     tc.tile_pool(name="ep1", bufs=2) as ep:
                    for g in range(NG):
                        gt = gtp.tile([P, G, ROW1], BF)
                        nc.gpsimd.dma_gather(gt[:, 0:G // 2, :], t1a[:, :],
                                             i1lo[:, g * IW:(g + 1) * IW],
                                             HLOW, HLOW, ROW1,
                                             single_packet=single_packet,
                                             queue_num=0 % nq)
                        nc.gpsimd.dma_gather(gt[:, G // 2:G, :], t1b[:, :],
                                             i1hi[:, g * IW:(g + 1) * IW],
                                             HLOW, HLOW, ROW1,
                                             single_packet=single_packet,
                                             queue_num=1 % nq)
                        if l1parts >= 2:
                            adg = adp.tile([P, G, cfg.ADROW], BF)
                            nc.gpsimd.dma_gather(adg[:, :, :], adt[:, :],
                                                 iad[:, g * AW:(g + 1) * AW],
                                                 GE, GE, cfg.ADROW,
                                                 single_packet=single_packet,
                                                 queue_num=2 % nq)
                        if l1parts >= 3:
                            ind = indp.tile([P, GE], BF)
                            nc.sync.dma_start(out=ind[:], in_=ind_in[:, g * GE:(g + 1) * GE])
                        if l1parts >= 4:
                            # p = exp(leaky(as + ad)) in-place in the as slot
                            as_v = gt[:, :, HC1:HC1 + H1]
                            nc.vector.tensor_tensor(out=as_v, in0=as_v,
                                                    in1=adg[:, :, H1:2 * H1],
                                                    op=mybir.AluOpType.add)
                            nc.vector.scalar_tensor_tensor(
                                out=as_v, in0=as_v, scalar=0.2, in1=as_v,
                                op0=mybir.AluOpType.mult, op1=mybir.AluOpType.max)
                            nc.scalar.activation(out=as_v, in_=as_v,
                                                 func=mybir.ActivationFunctionType.Exp)
                        if l1parts >= 5:
                            # msg *= p (broadcast over channels)
                            h_v = gt[:, :, 0:HC1].rearrange("p t (h c) -> p t h c", c=cfg.HID)
                            p_v = gt[:, :, HC1:HC1 + H1].unsqueeze(-1).broadcast_to(
                                [P, G, H1, cfg.HID])
                            nc.vector.tensor_tensor(out=h_v, in0=h_v, in1=p_v,
                                                    op=mybir.AluOpType.mult)
                        if l1parts >= 6:
                            ps = psp.tile([P, HC1 + H1], F32, space="PSUM")
                            for t in range(G):
                                nc.tensor.matmul(out=ps[:], lhsT=ind[:, t * P:(t + 1) * P],
                                                 rhs=gt[:, t, 0:HC1 + H1],
                                                 start=(t == 0), stop=(t == G - 1))
                        if l1parts < 7:
                            continue
                        den = ep.tile([P, H1], F32)
                        nc.vector.tensor_scalar_add(out=den[:], in0=ps[:, HC1:],
                                                    scalar1=1e-30)
                        rec = ep.tile([P, H1], F32)
                        nc.vector.reciprocal(out=rec[:], in_=den[:])
                        o1 = ep.tile([P, HC1], BF)
                        for h in range(H1):
                            nc.scalar.activation(
                                out=o1[:, h * cfg.HID:(h + 1) * cfg.HID],
                                in_=ps[:, h * cfg.HID:(h + 1) * cfg.HID],
                                func=mybir.ActivationFunctionType.Relu,
                                scale=rec[:, h:h + 1])
                        h2 = h2pp.tile([P, OUT + 2], F32, space="PSUM")
                        for k in range(HC1 // P):
                            tp = tpp.tile([P, P], BF, space="PSUM")
                            nc.tensor.transpose(out=tp[:], in_=o1[:, k * P:(k + 1) * P],
                                                identity=ident[:])
                            tt = ep.tile([P, P], BF, tag="tt")
                            nc.vector.tensor_copy(out=tt[:], in_=tp[:])
                            nc.tensor.matmul(out=h2[:], lhsT=tt[:], rhs=w2e[k][:],
                                             start=(k == 0), stop=(k == HC1 // P - 1))
                        row2 = ep.tile([P, ROW2], BF, tag="row2")
                        nc.scalar.activation(out=row2[:, 0:OUT + 2], in_=h2[:],
                                             func=mybir.ActivationFunctionType.Copy)
                        nc.vector.memset(row2[:, OUT + 2:], 0.0)
                        nc.gpsimd.indirect_dma_start(
                            out=t2s[:, :],
                            out_offset=bass.IndirectOffsetOnAxis(
                                ap=scat[:, g:g + 1], axis=0),
                            in_=row2[:], in_offset=None)

            # ---------------- exchange ----------------
            def phase_ag():
                nc.gpsimd.collective_compute(
                    "AllGather", mybir.AluOpType.bypass,
                    replica_groups=[list(range(NC))],
                    ins=[t2s[:]], outs=[t2f[:]])

            # ---------------- layer-2 edge phase ----------------
            def phase_l2():
                HALF2 = (NC // 2) * SHARDR
                with tc.tile_pool(name="g2", bufs=3) as g2p, \
                     tc.tile_pool(name="ad2", bufs=3) as ad2p, \
                     tc.tile_pool(name="indp2", bufs=3) as indp2, \
                     tc.tile_pool(name="ps2", bufs=2, space="PSUM") as ps2p, \
                     tc.tile_pool(name="ep2", bufs=2) as ep2:
                    for g in range(NG):
                        g2 = g2p.tile([P, G, ROW2], BF)
                        nc.gpsimd.dma_gather(g2[:, 0:G // 2, :], t2f[0:HALF2, :],
                                             i2lo[:, g * IW:(g + 1) * IW],
                                             HLOW, HLOW, ROW2,
                                             single_packet=single_packet,
                                             queue_num=0 % nq)
                        nc.gpsimd.dma_gather(g2[:, G // 2:G, :], t2f[HALF2:, :],
                                             i2hi[:, g * IW:(g + 1) * IW],
                                             HLOW, HLOW, ROW2,
                                             single_packet=single_packet,
                                             queue_num=1 % nq)
                        ad2 = ad2p.tile([P, G, ROW2], BF)
                        nc.gpsimd.dma_gather(ad2[:, :, :], t2s[:, :],
                                             iad[:, g * AW:(g + 1) * AW],
                                             GE, GE, ROW2,
                                             single_packet=single_packet,
                                             queue_num=2 % nq)
                        ind = indp2.tile([P, GE], BF, tag="ind2")
                        nc.sync.dma_start(out=ind[:], in_=ind_in[:, g * GE:(g + 1) * GE])
                        as_v = g2[:, :, OUT:OUT + 1]
                        nc.vector.tensor_tensor(out=as_v, in0=as_v,
                                                in1=ad2[:, :, OUT + 1:OUT + 2],
                                                op=mybir.AluOpType.add)
                        nc.vector.scalar_tensor_tensor(
                            out=as_v, in0=as_v, scalar=0.2, in1=as_v,
                            op0=mybir.AluOpType.mult, op1=mybir.AluOpType.max)
                        nc.scalar.activation(out=as_v, in_=as_v,
                                             func=mybir.ActivationFunctionType.Exp)
                        h_v = g2[:, :, 0:OUT]
                        p_v = g2[:, :, OUT:OUT + 1].broadcast_to([P, G, OUT])
                        nc.vector.tensor_tensor(out=h_v, in0=h_v, in1=p_v,
                                                op=mybir.AluOpType.mult)
                        ps2 = ps2p.tile([P, OUT + 1], F32, space="PSUM")
                        for t in range(G):
                            nc.tensor.matmul(out=ps2[:], lhsT=ind[:, t * P:(t + 1) * P],
                                             rhs=g2[:, t, 0:OUT + 1],
                                             start=(t == 0), stop=(t == G - 1))
                        den = ep2.tile([P, 1], F32, tag="den2")
                        nc.vector.tensor_scalar_add(out=den[:], in0=ps2[:, OUT:],
                                                    scalar1=1e-30)
                        rec = ep2.tile([P, 1], F32, tag="rec2")
                        nc.vector.reciprocal(out=rec[:], in_=den[:])
                        o2 = ep2.tile([P, OUT], F32, tag="o2")
                        nc.scalar.activation(out=o2[:], in_=ps2[:, 0:OUT],
                                             func=mybir.ActivationFunctionType.Copy,
                                             scale=rec[:, 0:1])
                        nc.gpsimd.indirect_dma_start(
                            out=out_sh[:, :],
                            out_offset=bass.IndirectOffsetOnAxis(
                                ap=scat[:, g:g + 1], axis=0),
                            in_=o2[:], in_offset=None)


            schedule = ([phases] * repeat if not isinstance(phases[0], (tuple, list))
                        else phases)
            for _ph in schedule:
                if "node" in _ph:
                    phase_node()
                if "l1" in _ph:
                    phase_l1()
                if "ag" in _ph:
                    phase_ag()
                if "l2" in _ph:
                    phase_l2()

    nc.compile()
    return nc


# --------------------------------------------------------------------------
# entry point
# --------------------------------------------------------------------------
_cache = {}


def _build_in_maps(inputs, cfg, pp):
    import ml_dtypes
    x = np.asarray(inputs["x"], np.float32)
    assert not np.asarray(inputs["b1"]).any() and not np.asarray(inputs["b2"]).any(), \
        "nonzero biases not supported by this kernel build"
    W1e, W2e = make_weights(inputs, cfg)
    xp = np.zeros((cfg.NPAD, cfg.IN), np.float32)
    xp[:cfg.N] = x
    xT = np.ascontiguousarray(xp.T)                       # [IN=128, NPAD]
    w2e_s = np.zeros((2, P, cfg.OUT + 2), np.float32)
    w2e_s[0] = W2e[:P]
    w2e_s[1] = W2e[P:]
    in_maps = []
    for c in range(cfg.ncores):
        pc = pp["per_core"][c]
        in_maps.append({
            "xT": xT, "w1e": W1e,
            "w2e": w2e_s.astype(ml_dtypes.bfloat16),
            "idx1lo": pc["idx1lo"], "idx1hi": pc["idx1hi"],
            "idx2lo": pc["idx2lo"], "idx2hi": pc["idx2hi"],
            "idxad": pc["idxad"], "ind": pc["ind"], "scat": pc["scat"],
            "adt_sidx": pp["adt_sidx"][c],
        })
    return in_maps


def kernel(**inputs):
    from concourse.bass_utils import run_bass_kernel_spmd
    cfg = FULL
    ei = np.asarray(inputs["edge_index"])
    pp = prep_edges(ei, cfg)
    key = (cfg.N, pp["NG"], pp["SHARDR"])
    if key not in _cache:
        _cache[key] = build_kernel(cfg, pp["NG"], pp["SHARDR"])
    nc = _cache[key]
    in_maps = _build_in_maps(inputs, cfg, pp)
    res = run_bass_kernel_spmd(nc, in_maps, list(range(cfg.ncores)))
    out = np.zeros((cfg.N, cfg.OUT), np.float32)
    for c, (n0, n1) in enumerate(pp["node_ranges"]):
        out[n0:n1] = res.results[c]["out_shard"][:n1 - n0]
    return out


# --------------------------------------------------------------------------
# numpy simulation of the exact device dataflow (for testing)
# --------------------------------------------------------------------------
def numpy_sim(inputs, cfg=None, use_bf16=True):
    import ml_dtypes

    def cast(a):
        if not use_bf16:
            return np.asarray(a, np.float32)
        return np.asarray(a, np.float32).astype(ml_dtypes.bfloat16).astype(np.float32)

    cfg = cfg or FULL
    G, GE, HLOW = cfg.G, cfg.GE, cfg.HLOW
    IW, AW = HLOW // 16, GE // 16
    pp = prep_edges(np.asarray(inputs["edge_index"]), cfg)
    NG, SHARDR, maxn = pp["NG"], pp["SHARDR"], pp["maxn"]
    NC, HC1, H1, OUT, HID = cfg.ncores, cfg.HC1, cfg.H1, cfg.OUT, cfg.HID
    W1e, W2e = make_weights(inputs, cfg)
    xp = np.zeros((cfg.NPAD, cfg.IN), np.float32)
    xp[:cfg.N] = np.asarray(inputs["x"], np.float32)
    hrow = cast(xp @ W1e)                                 # [NPAD, 264]
    t1 = np.zeros((cfg.NPAD, cfg.ROW1), np.float32)
    t1[:, :HC1 + 2 * H1] = hrow
    t1a, t1b = t1[:cfg.NHALF], t1[cfg.NHALF:]
    W2c = cast(W2e)

    def unwrap(a):      # [128, S] -> flat [S*16]
        return np.ascontiguousarray(a[:16].T).reshape(-1)

    t2f = np.zeros((NC * SHARDR, cfg.ROW2), np.float32)
    adts, out_shards = [], []
    for c in range(NC):
        pc = pp["per_core"][c]
        n0, n1 = pc["n0"], pc["n1"]
        adt = np.zeros((SHARDR, 2 * H1), np.float32)
        adt[:n1 - n0] = hrow[n0:n1, HC1:]
        adts.append(adt)
        t2sh = np.zeros((SHARDR, cfg.ROW2), np.float32)
        for g in range(NG):
            ilo = unwrap(pc["idx1lo"][:, g * IW:(g + 1) * IW])
            ihi = unwrap(pc["idx1hi"][:, g * IW:(g + 1) * IW])
            iad = unwrap(pc["idxad"][:, g * AW:(g + 1) * AW])
            gt = np.concatenate([t1a[ilo], t1b[ihi]])     # [GE, ROW1] flat order
            adg = adts[c][iad]                            # [GE, 8]
            t = gt[:, HC1:HC1 + H1] + adg[:, H1:]
            p = cast(np.exp(np.maximum(t, 0.2 * t)))
            msg = cast(gt[:, :HC1] * np.repeat(p, HID, axis=1))
            indg = pc["ind"][:, g * GE:(g + 1) * GE]  # [P, GE]
            # device layout: ind[p, t*128+m]; edge flat j=(t*128+p)
            ps = np.zeros((P, HC1 + H1), np.float32)
            for tt_ in range(G):
                lhsT = indg[:, tt_ * P:(tt_ + 1) * P].astype(np.float32)
                rhs = np.concatenate([msg[tt_ * P:(tt_ + 1) * P],
                                      p[tt_ * P:(tt_ + 1) * P]], axis=1)
                ps += lhsT.T @ rhs
            rec = 1.0 / (ps[:, HC1:] + 1e-30)
            o1 = cast(np.maximum(ps[:, :HC1], 0.0) *
                      np.repeat(rec, HID, axis=1))
            h2 = np.zeros((P, cfg.ROW2), np.float32)
            h2[:, :OUT + 2] = cast(o1 @ W2c)
            t2sh[pc["scat"][:, g]] = h2
        t2f[c * SHARDR:(c + 1) * SHARDR] = t2sh
        out_shards.append(np.zeros((SHARDR, OUT), np.float32))

    HALF2 = (NC // 2) * SHARDR
    out = np.zeros((cfg.N, OUT), np.float32)
    for c in range(NC):
        pc = pp["per_core"][c]
        t2sh = t2f[c * SHARDR:(c + 1) * SHARDR]
        for g in range(NG):
            ilo = unwrap(pc["idx2lo"][:, g * IW:(g + 1) * IW])
            ihi = unwrap(pc["idx2hi"][:, g * IW:(g + 1) * IW])
            iad = unwrap(pc["idxad"][:, g * AW:(g + 1) * AW])
            gt = np.concatenate([t2f[:HALF2][ilo], t2f[HALF2:][ihi]])
            ad2 = t2sh[iad]
            t = gt[:, OUT:OUT + 1] + ad2[:, OUT + 1:OUT + 2]
            p = cast(np.exp(np.maximum(t, 0.2 * t)))
            msg = cast(gt[:, :OUT] * p)
            indg = pc["ind"][:, g * GE:(g + 1) * GE]
            ps = np.zeros((P, OUT + 1), np.float32)
            for tt_ in range(G):
                lhsT = indg[:, tt_ * P:(tt_ + 1) * P].astype(np.float32)
                rhs = np.concatenate([msg[tt_ * P:(tt_ + 1) * P],
                                      p[tt_ * P:(tt_ + 1) * P]], axis=1)
                ps += lhsT.T @ rhs
            rec = 1.0 / (ps[:, OUT:] + 1e-30)
            out_shards[c][pc["scat"][:, g]] = ps[:, :OUT] * rec
        n0, n1 = pc["n0"], pc["n1"]
        out[n0:n1] = out_shards[c][:n1 - n0]
    return out

